# revision 24
# baseline (speedup 1.0000x reference)
"""HiRA layer (rank-modulated linear) Trainium2 kernel.

Computes out = x @ (W * (1 + A^T B^T)^T)^T + bias for
x:[4,2048,4096] f32, W:[4096,4096], A:[16,4096], B:[4096,16], bias:[4096].

Sharding: pure column-parallel over 8 NeuronCores — each core owns a
512-wide slice of out_features, x replicated (per the tensor-parallel
hint).  Per core:
  1. builds its adapted-weight shard on device:
     P'[i,o] = sum_r A_aug[r,i] * B_aug^T[r,o]   (ones-row augmentation
     folds the +1 into the matmul), then AWT[i,o] = W^T[i,o] * P'[i,o]
     cast to bf16, resident in SBUF.  W is shipped bf16, per-chunk
     (128KB DMAs) so the first chunks land before the PE needs them.
  2. streams x tiles (host pre-blocked to [m, p=i, k, t=tok] bf16)
     through the PE: psum[tok, o] accumulates 32 chunk matmuls, bias is
     added on DVE during the PSUM->SBUF copy, out DMA'd f32.

Schedule: a scratch-matmul warmup bridges the PE from engine boot to
the first W chunk so the HAM clock gate opens (1.2 -> 2.4 GHz) and
never re-arms; the 32 modulation chunks are interleaved with the m=0,
m=1 and m=2 main accumulations (skewed 3/6/9 chunks back) so PE demand
exceeds the W arrival rate; m=3..63 is a pure back-to-back matmul
stream at the PE issue-rate floor.  The first x tiles are DMA'd in
quarters/halves so the interleaved mains are never gated on a 1MB
transfer stuck behind the round-robin DMA rings.

Host side only reshapes/transposes/casts and slices shards; every FLOP
of the reference computation happens on device.
"""

import sys

for _p in ("/opt/trn_rl_repo",):
    if _p not in sys.path:
        sys.path.insert(0, _p)

import numpy as np
import ml_dtypes

BF16 = ml_dtypes.bfloat16

# problem shape (hardcoded per contract)
B, S, IN, OUT, R = 4, 2048, 4096, 4096, 16
TOK = B * S            # 8192 tokens, all on every core
OB = 8                 # out-feature slices = 8 cores
OQ = OUT // OB         # 512 out features per core
MT = TOK // 128        # 64 token tiles
KT = IN // 128         # 32 contraction chunks
N_CORES = 8

TRACE = False          # test.py sets True to capture NTFF exec time
LAST_RESULT = None     # BassKernelResults of the most recent run

_NC_CACHE = None


def _build_nc():
    import concourse.bass as bass
    import concourse.bacc as bacc
    import concourse.mybir as mybir
    from concourse import tile

    f32 = mybir.dt.float32
    bf16 = mybir.dt.bfloat16

    nc = bacc.Bacc(
        "TRN2", target_bir_lowering=False, debug=False, num_devices=N_CORES
    )

    XB = nc.dram_tensor("xb", [MT, 128, KT, 128], bf16, kind="ExternalInput")
    # W pre-blocked on host as [group, p, chunk-in-group, o] so each
    # group DMA moves 8KB-contiguous per-partition rows (2KB+ per line
    # is needed for full DMA throughput; chunk-major layout would give
    # 1KB lines and halve it).
    WG = 4
    WT = nc.dram_tensor(
        "wt", [WG, 128, KT // WG, OQ], bf16, kind="ExternalInput"
    )
    AAUG = nc.dram_tensor("a_aug", [R + 1, IN], bf16, kind="ExternalInput")
    BTAUG = nc.dram_tensor("bt_aug", [R + 1, OQ], bf16, kind="ExternalInput")
    BIASB = nc.dram_tensor("bias_b", [128, OQ], bf16, kind="ExternalInput")
    OUTP = nc.dram_tensor("out", [MT, 128, OQ], f32, kind="ExternalOutput")

    NSTART = 4            # m-tiles interleaved with modulation
    SKEW = {0: 3, 1: 6, 2: 9, 3: 12}
    KPG = KT // WG        # W chunks per group

    with tile.TileContext(nc) as tc:
        with (
            tc.tile_pool(name="const", bufs=1) as const,
            tc.tile_pool(name="awt", bufs=1) as awtp,
            tc.tile_pool(name="wtld", bufs=4) as wtld,
            tc.tile_pool(name="xb", bufs=5) as xbp,
            tc.tile_pool(name="ob", bufs=3) as obp,
            tc.tile_pool(name="ppsum", bufs=3, space=bass.MemorySpace.PSUM) as ppp,
            tc.tile_pool(name="opsum", bufs=5, space=bass.MemorySpace.PSUM) as opp,
        ):
            a_t = const.tile([R + 1, IN], bf16)
            bt_t = const.tile([R + 1, OQ], bf16)
            nc.sync.dma_start(out=a_t[:], in_=AAUG[:])
            nc.sync.dma_start(out=bt_t[:], in_=BTAUG[:])

            # W group DMAs (1MB each, 8KB per-partition rows) interleaved
            # in emission order with the halved first x tiles so the
            # round-robin DMA rings deliver both streams in lockstep with
            # PE demand during the modulation phase.
            wt_g = []
            for g in range(WG):
                wg = wtld.tile(
                    [128, KPG, OQ], bf16, tag="wt", name=f"wtg{g}"
                )
                wt_g.append(wg)
            xb_early = {}
            for m in range(NSTART):
                xb_early[m] = xbp.tile(
                    [128, KT, 128], bf16, tag="xb", name=f"xbe{m}"
                )

            nc.sync.dma_start(out=wt_g[0][:], in_=WT[0])
            nc.sync.dma_start(
                out=xb_early[0][:, 0:16, :], in_=XB[0, :, 0:16, :]
            )
            nc.sync.dma_start(out=wt_g[1][:], in_=WT[1])
            nc.sync.dma_start(
                out=xb_early[0][:, 16:32, :], in_=XB[0, :, 16:32, :]
            )
            nc.sync.dma_start(
                out=xb_early[1][:, 0:16, :], in_=XB[1, :, 0:16, :]
            )
            nc.sync.dma_start(out=wt_g[2][:], in_=WT[2])
            nc.sync.dma_start(
                out=xb_early[1][:, 16:32, :], in_=XB[1, :, 16:32, :]
            )
            nc.sync.dma_start(out=wt_g[3][:], in_=WT[3])
            nc.sync.dma_start(out=xb_early[2][:], in_=XB[2])
            nc.sync.dma_start(out=xb_early[3][:], in_=XB[3])
            bias_t = const.tile([128, OQ], bf16)
            nc.sync.dma_start(out=bias_t[:], in_=BIASB[:])

            # adapted weight, bf16, resident: [p=i%128, k=i//128, o]
            awt = awtp.tile([128, KT, OQ], bf16)

            # PE warmup: scratch matmuls bridge from engine boot (~8us) to
            # the first W chunk so the HAM SHORT window sees sustained busy
            # and un-gates the clock before real work starts.
            wu_l = const.tile([128, 128], bf16)
            wu_r = const.tile([128, 512], bf16)
            nc.vector.memset(wu_l[:], 0.0)
            nc.vector.memset(wu_r[:], 0.0)

            def scratch_mm(n=1):
                for _ in range(n):
                    wu_p = ppp.tile([128, 512], f32, tag="pp", name="wu_p")
                    nc.tensor.matmul(
                        wu_p[:], wu_l[:], wu_r[:], start=True, stop=True
                    )

            scratch_mm(22)

            def mod_chunk(k):
                """AWT[:, k, :] = (A_aug^T @ B_aug^T) * W^T for one chunk."""
                pp_t = ppp.tile([128, OQ], f32, tag="pp", name="pp_t")
                nc.tensor.matmul(
                    pp_t[:],
                    a_t[:, k * 128:(k + 1) * 128],
                    bt_t[:],
                    start=True,
                    stop=True,
                )
                nc.vector.tensor_mul(
                    awt[:, k, :], pp_t[:], wt_g[k // KPG][:, k % KPG, :]
                )

            po = {
                m: opp.tile([128, OQ], f32, tag="po", name=f"po{m}")
                for m in range(NSTART)
            }

            def main_mm(m, xb_tile, j):
                nc.tensor.matmul(
                    po[m][:],
                    xb_tile[:, j, :],
                    awt[:, j, :],
                    start=(j == 0),
                    stop=(j == KT - 1),
                )

            # Startup: modulation interleaved with m=0..2 accumulation,
            # each skewed so the DVE product is ready before the PE reads
            # it and PE demand outpaces W chunk arrival.
            for k in range(KT + SKEW[NSTART - 1]):
                if k < KT:
                    mod_chunk(k)
                for m in range(NSTART):
                    j = k - SKEW[m]
                    if 0 <= j < KT:
                        main_mm(m, xb_early[m], j)

            def drain(m, po_t):
                o_t = obp.tile([128, OQ], f32, tag="ot", name="o_t")
                nc.vector.tensor_add(o_t[:], po_t[:], bias_t[:])
                nc.sync.dma_start(out=OUTP[m, :, :], in_=o_t[:])

            for m in range(NSTART):
                drain(m, po[m])

            # Steady state: pure back-to-back matmul stream.
            for m in range(NSTART, MT):
                xb_t = xbp.tile([128, KT, 128], bf16, tag="xb", name="xb_t")
                nc.sync.dma_start(out=xb_t[:], in_=XB[m])
                po_t = opp.tile([128, OQ], f32, tag="po", name="po_t")
                for k in range(KT):
                    nc.tensor.matmul(
                        po_t[:], xb_t[:, k, :], awt[:, k, :],
                        start=(k == 0), stop=(k == KT - 1),
                    )
                drain(m, po_t)

    nc.compile()
    return nc


def _get_nc():
    global _NC_CACHE
    if _NC_CACHE is None:
        _NC_CACHE = _build_nc()
    return _NC_CACHE


def kernel(x, weight, bias, lora_A, lora_B):
    global LAST_RESULT
    from concourse.bass_utils import run_bass_kernel_spmd

    x = np.asarray(x, dtype=np.float32)
    weight = np.asarray(weight, dtype=np.float32)
    bias = np.asarray(bias, dtype=np.float32)
    lora_A = np.asarray(lora_A, dtype=np.float32)
    lora_B = np.asarray(lora_B, dtype=np.float32)

    x2 = x.reshape(TOK, IN)

    # x blocked: [m, p=i%128, k=i//128, t=tok%128] bf16, replicated
    xb = x2.reshape(MT, 128, KT, 128).transpose(0, 3, 2, 1)  # [m,p,k,t]
    xb = np.ascontiguousarray(xb.astype(BF16))

    a_aug = np.concatenate(
        [lora_A, np.ones((1, IN), np.float32)], axis=0
    ).astype(BF16)

    in_maps = []
    for ob in range(OB):
        osl = slice(ob * OQ, (ob + 1) * OQ)
        wq = weight[osl]                                   # [OQ, IN]
        wts = np.ascontiguousarray(
            wq.T.reshape(4, KT // 4, 128, OQ)
            .transpose(0, 2, 1, 3)
            .astype(BF16)
        )
        bq = lora_B[osl]                                   # [OQ, R]
        bts = np.ascontiguousarray(
            np.concatenate(
                [bq.T, np.ones((1, OQ), np.float32)], axis=0
            ).astype(BF16)
        )
        bias_b = np.ascontiguousarray(
            np.tile(bias[osl][None, :], (128, 1)).astype(BF16)
        )
        in_maps.append(
            {
                "xb": xb,
                "wt": wts,
                "a_aug": a_aug,
                "bt_aug": bts,
                "bias_b": bias_b,
            }
        )

    nc = _get_nc()
    res = run_bass_kernel_spmd(
        nc, in_maps, core_ids=list(range(N_CORES)), trace=TRACE
    )
    LAST_RESULT = res

    # reassemble: out[c] is [MT, 128, OQ] -> [TOK, OQ]; concat out slices
    cols = [
        res.results[ob]["out"].reshape(TOK, OQ) for ob in range(OB)
    ]
    full = np.concatenate(cols, axis=1).reshape(B, S, OUT)
    return full


# revision 31
# speedup vs baseline: 1.0060x; 1.0060x over previous
"""HiRA layer (rank-modulated linear) Trainium2 kernel.

Computes out = x @ (W * (1 + A^T B^T)^T)^T + bias for
x:[4,2048,4096] f32, W:[4096,4096], A:[16,4096], B:[4096,16], bias:[4096].

Sharding: pure column-parallel over 8 NeuronCores — each core owns a
512-wide slice of out_features, x replicated (per the tensor-parallel
hint).  Per core:
  1. builds its adapted-weight shard on device:
     P'[i,o] = sum_r A_aug[r,i] * B_aug^T[r,o]   (ones-row augmentation
     folds the +1 into the matmul), then AWT[i,o] = W^T[i,o] * P'[i,o]
     cast to bf16, resident in SBUF.  W is shipped bf16, per-chunk
     (128KB DMAs) so the first chunks land before the PE needs them.
  2. streams x tiles (host pre-blocked to [m, p=i, k, t=tok] bf16)
     through the PE: psum[tok, o] accumulates 32 chunk matmuls, bias is
     added on DVE during the PSUM->SBUF copy, out DMA'd f32.

Schedule: a scratch-matmul warmup bridges the PE from engine boot to
the first W chunk so the HAM clock gate opens (1.2 -> 2.4 GHz) and
never re-arms; the 32 modulation chunks are interleaved with the m=0,
m=1 and m=2 main accumulations (skewed 3/6/9 chunks back) so PE demand
exceeds the W arrival rate; m=3..63 is a pure back-to-back matmul
stream at the PE issue-rate floor.  The first x tiles are DMA'd in
quarters/halves so the interleaved mains are never gated on a 1MB
transfer stuck behind the round-robin DMA rings.

Host side only reshapes/transposes/casts and slices shards; every FLOP
of the reference computation happens on device.
"""

import sys

for _p in ("/opt/trn_rl_repo",):
    if _p not in sys.path:
        sys.path.insert(0, _p)

import numpy as np
import ml_dtypes

BF16 = ml_dtypes.bfloat16

# problem shape (hardcoded per contract)
B, S, IN, OUT, R = 4, 2048, 4096, 4096, 16
TOK = B * S            # 8192 tokens, all on every core
OB = 8                 # out-feature slices = 8 cores
OQ = OUT // OB         # 512 out features per core
MT = TOK // 128        # 64 token tiles
KT = IN // 128         # 32 contraction chunks
N_CORES = 8

TRACE = False          # test.py sets True to capture NTFF exec time
LAST_RESULT = None     # BassKernelResults of the most recent run

_NC_CACHE = None


def _build_nc():
    import concourse.bass as bass
    import concourse.bacc as bacc
    import concourse.mybir as mybir
    from concourse import tile

    f32 = mybir.dt.float32
    bf16 = mybir.dt.bfloat16

    nc = bacc.Bacc(
        "TRN2", target_bir_lowering=False, debug=False, num_devices=N_CORES
    )

    XB = nc.dram_tensor("xb", [MT, 128, KT, 128], bf16, kind="ExternalInput")
    # W pre-blocked on host partition-major [p, k, o]: any chunk range
    # is then a contiguous per-partition row slice, so it can stream
    # into one resident SBUF tile in graduated pieces (small first) and
    # subtile deps gate each modulation chunk on just its piece.
    WT = nc.dram_tensor("wt", [128, KT, OQ], bf16, kind="ExternalInput")
    AAUG = nc.dram_tensor("a_aug", [R + 1, IN], bf16, kind="ExternalInput")
    BTAUG = nc.dram_tensor("bt_aug", [R + 1, OQ], bf16, kind="ExternalInput")
    BIASB = nc.dram_tensor("bias_b", [128, OQ], bf16, kind="ExternalInput")
    OUTP = nc.dram_tensor("out", [MT, 128, OQ], f32, kind="ExternalOutput")

    NSTART = 4            # m-tiles interleaved with modulation
    SKEW = {0: 3, 1: 6, 2: 9, 3: 12}
    # W piece boundaries: small pieces first so the first modulation
    # chunks are not gated on a megabyte-scale transfer.
    WPIECES = [(0, 2), (2, 4), (4, 8), (8, 16), (16, 24), (24, 32)]

    with tile.TileContext(nc) as tc:
        with (
            tc.tile_pool(name="const", bufs=1) as const,
            tc.tile_pool(name="awt", bufs=1) as awtp,
            tc.tile_pool(name="wtld", bufs=1) as wtld,
            tc.tile_pool(name="xb", bufs=5) as xbp,
            tc.tile_pool(name="ob", bufs=3) as obp,
            tc.tile_pool(name="ppsum", bufs=3, space=bass.MemorySpace.PSUM) as ppp,
            tc.tile_pool(name="opsum", bufs=5, space=bass.MemorySpace.PSUM) as opp,
        ):
            a_t = const.tile([R + 1, IN], bf16)
            bt_t = const.tile([R + 1, OQ], bf16)
            nc.sync.dma_start(out=a_t[:], in_=AAUG[:])
            nc.sync.dma_start(out=bt_t[:], in_=BTAUG[:])

            # W streams into one resident tile in graduated pieces,
            # interleaved in emission order with the halved first x tiles
            # so the round-robin DMA rings deliver both streams in
            # lockstep with PE demand during the modulation phase.
            wt_all = wtld.tile([128, KT, OQ], bf16)
            xb_early = {}
            for m in range(NSTART):
                xb_early[m] = xbp.tile(
                    [128, KT, 128], bf16, tag="xb", name=f"xbe{m}"
                )

            def dma_w(p):
                k0, k1 = WPIECES[p]
                nc.sync.dma_start(
                    out=wt_all[:, k0:k1, :], in_=WT[:, k0:k1, :]
                )

            dma_w(0)
            nc.sync.dma_start(
                out=xb_early[0][:, 0:16, :], in_=XB[0, :, 0:16, :]
            )
            dma_w(1)
            nc.sync.dma_start(
                out=xb_early[0][:, 16:32, :], in_=XB[0, :, 16:32, :]
            )
            dma_w(2)
            nc.sync.dma_start(
                out=xb_early[1][:, 0:16, :], in_=XB[1, :, 0:16, :]
            )
            dma_w(3)
            nc.sync.dma_start(
                out=xb_early[1][:, 16:32, :], in_=XB[1, :, 16:32, :]
            )
            dma_w(4)
            nc.sync.dma_start(out=xb_early[2][:], in_=XB[2])
            dma_w(5)
            nc.sync.dma_start(out=xb_early[3][:], in_=XB[3])
            bias_t = const.tile([128, OQ], bf16)
            nc.sync.dma_start(out=bias_t[:], in_=BIASB[:])

            # adapted weight, bf16, resident: [p=i%128, k=i//128, o]
            awt = awtp.tile([128, KT, OQ], bf16)

            # PE warmup: scratch matmuls bridge from engine boot (~8us) to
            # the first W chunk so the HAM SHORT window sees sustained busy
            # and un-gates the clock before real work starts.
            wu_l = const.tile([128, 128], bf16)
            wu_r = const.tile([128, 512], bf16)
            nc.vector.memset(wu_l[:], 0.0)
            nc.vector.memset(wu_r[:], 0.0)

            def scratch_mm(n=1):
                for _ in range(n):
                    wu_p = ppp.tile([128, 512], f32, tag="pp", name="wu_p")
                    nc.tensor.matmul(
                        wu_p[:], wu_l[:], wu_r[:], start=True, stop=True
                    )

            scratch_mm(16)

            def mod_chunk(k):
                """AWT[:, k, :] = (A_aug^T @ B_aug^T) * W^T for one chunk."""
                pp_t = ppp.tile([128, OQ], f32, tag="pp", name="pp_t")
                nc.tensor.matmul(
                    pp_t[:],
                    a_t[:, k * 128:(k + 1) * 128],
                    bt_t[:],
                    start=True,
                    stop=True,
                )
                nc.vector.tensor_mul(awt[:, k, :], pp_t[:], wt_all[:, k, :])

            po = {
                m: opp.tile([128, OQ], f32, tag="po", name=f"po{m}")
                for m in range(NSTART)
            }

            def main_mm(m, xb_tile, j):
                nc.tensor.matmul(
                    po[m][:],
                    xb_tile[:, j, :],
                    awt[:, j, :],
                    start=(j == 0),
                    stop=(j == KT - 1),
                )

            # Startup: modulation interleaved with m=0..2 accumulation,
            # each skewed so the DVE product is ready before the PE reads
            # it and PE demand outpaces W chunk arrival.
            for k in range(KT + SKEW[NSTART - 1]):
                if k < KT:
                    mod_chunk(k)
                for m in range(NSTART):
                    j = k - SKEW[m]
                    if 0 <= j < KT:
                        main_mm(m, xb_early[m], j)

            def drain(m, po_t):
                o_t = obp.tile([128, OQ], f32, tag="ot", name="o_t")
                nc.vector.tensor_add(o_t[:], po_t[:], bias_t[:])
                nc.sync.dma_start(out=OUTP[m, :, :], in_=o_t[:])

            for m in range(NSTART):
                drain(m, po[m])

            # Steady state: pure back-to-back matmul stream.
            for m in range(NSTART, MT):
                xb_t = xbp.tile([128, KT, 128], bf16, tag="xb", name="xb_t")
                nc.sync.dma_start(out=xb_t[:], in_=XB[m])
                po_t = opp.tile([128, OQ], f32, tag="po", name="po_t")
                for k in range(KT):
                    nc.tensor.matmul(
                        po_t[:], xb_t[:, k, :], awt[:, k, :],
                        start=(k == 0), stop=(k == KT - 1),
                    )
                drain(m, po_t)

    nc.compile()
    return nc


def _get_nc():
    global _NC_CACHE
    if _NC_CACHE is None:
        _NC_CACHE = _build_nc()
    return _NC_CACHE


def kernel(x, weight, bias, lora_A, lora_B):
    global LAST_RESULT
    from concourse.bass_utils import run_bass_kernel_spmd

    x = np.asarray(x, dtype=np.float32)
    weight = np.asarray(weight, dtype=np.float32)
    bias = np.asarray(bias, dtype=np.float32)
    lora_A = np.asarray(lora_A, dtype=np.float32)
    lora_B = np.asarray(lora_B, dtype=np.float32)

    x2 = x.reshape(TOK, IN)

    # x blocked: [m, p=i%128, k=i//128, t=tok%128] bf16, replicated
    xb = x2.reshape(MT, 128, KT, 128).transpose(0, 3, 2, 1)  # [m,p,k,t]
    xb = np.ascontiguousarray(xb.astype(BF16))

    a_aug = np.concatenate(
        [lora_A, np.ones((1, IN), np.float32)], axis=0
    ).astype(BF16)

    in_maps = []
    for ob in range(OB):
        osl = slice(ob * OQ, (ob + 1) * OQ)
        wq = weight[osl]                                   # [OQ, IN]
        wts = np.ascontiguousarray(
            wq.T.reshape(KT, 128, OQ).transpose(1, 0, 2).astype(BF16)
        )
        bq = lora_B[osl]                                   # [OQ, R]
        bts = np.ascontiguousarray(
            np.concatenate(
                [bq.T, np.ones((1, OQ), np.float32)], axis=0
            ).astype(BF16)
        )
        bias_b = np.ascontiguousarray(
            np.tile(bias[osl][None, :], (128, 1)).astype(BF16)
        )
        in_maps.append(
            {
                "xb": xb,
                "wt": wts,
                "a_aug": a_aug,
                "bt_aug": bts,
                "bias_b": bias_b,
            }
        )

    nc = _get_nc()
    res = run_bass_kernel_spmd(
        nc, in_maps, core_ids=list(range(N_CORES)), trace=TRACE
    )
    LAST_RESULT = res

    # reassemble: out[c] is [MT, 128, OQ] -> [TOK, OQ]; concat out slices
    cols = [
        res.results[ob]["out"].reshape(TOK, OQ) for ob in range(OB)
    ]
    full = np.concatenate(cols, axis=1).reshape(B, S, OUT)
    return full
